# revision 9
# baseline (speedup 1.0000x reference)
"""Trainium2 Bass kernel for nn_ConvAttention_34600256537137.

Math notes (validated against the reference):
  qkv = 1x1conv(x, w1)+b1 -> Q,K,V;  score = conv5x5(Q_s)+conv5x5(K_t)+b2;
  attn = softmax_t(score);  out = einsum(attn, V).
  Softmax over t is shift-invariant, so the Q-half of the score (constant in
  t), b2, and the K-path bias all cancel.  The computation collapses to:
    weff[ci,dy,dx] = sum_c w1K[c,ci] * w2K[c,dy,dx]        (host, tiny)
    sK[b,t,h,w]    = conv5x5_reflect(x[b,:,:,:,t], weff)
    e = exp(sK);  den = sum_t e
    out[b,o,h,w,s] = (sum_{ci,t} w1V[o,ci] * e * x) / den + b1V[o]
  (s-independent; normalization + S-broadcast + bias done on host)

Sharding: 8 cores = (b in {0,1}) x (4 chunks of 8 rows of H).

Perf structure (v5):
  - bf16 end-to-end on device (PSUM accumulates fp32); 4x PE rate vs fp32.
  - phase 1 (score conv): T[tap, pos] = weff^T @ slab in 14 contiguous
    512-col matmuls into 2-bank PSUM tiles; 7 PSUM->SBUF cast copies
    rotate over DVE/Act/Pool; slab free layout (row~, t, w~).
  - T bounced to DRAM in [tap, row~, t, w~] layout: the (row,t) partition
    pair merges to a single stride-36 dim, dy folds into the DMA offset and
    dx becomes a stride-(NPOS+1) dim -> the 25 shift-gathers collapse to
    10 DMAs (dy x row-half), interleaved with the 6 td writes so they
    pipeline behind the conv copies.
  - softmax split into row-halves: DVE tap-reduce, exp on Act, e bounced
    to DRAM in (t, row, w) order so ONE strided read with a 0-stride
    ci8-replication dim rebroadcasts e to all 128 partitions per half.
  - denominator via 2 accumulating indicator-matmuls, reciprocal on DVE,
    1/den rebroadcast via tiny DRAM bounce (off critical path).
  - V path: xattn = x_t * e_b (DVE, 2 chunks); contract (ci,t) on PE in 8
    bf16 matmuls; normalization folded into the PSUM->SBUF read.
  - output is [C, HW] only (S-broadcast on host): 64KB instead of 1MB.
"""

import sys

if "/opt/trn_rl_repo" not in sys.path:
    sys.path.insert(0, "/opt/trn_rl_repo")

import numpy as np
import ml_dtypes

BF16 = ml_dtypes.bfloat16

B, C, H, W, S = 2, 64, 32, 32, 16
KS, PAD = 5, 2
NCORES = 8
ROWS = H // 4            # output rows per core
SLAB_R = ROWS + 2 * PAD  # 12
SLAB_W = W + 2 * PAD     # 36
NTAP = KS * KS           # 25
NPOS = SLAB_R * SLAB_W * S  # 6912 slab positions per tap
HW = ROWS * W            # 256 output positions
RT = SLAB_W * S          # 576 = one row~'s (t, w~) block
CH = 512                 # matmul chunk (free cols); 6912 = 13*512 + 256

_MODULE = None


def _build_module():
    import concourse.bacc as bacc
    import concourse.bass as bass
    import concourse.tile as tile
    from concourse import mybir

    f32 = mybir.dt.float32
    bf16 = mybir.dt.bfloat16
    AF = mybir.ActivationFunctionType
    ALU = mybir.AluOpType
    nc = bacc.Bacc("TRN2", target_bir_lowering=False, debug=False, num_devices=NCORES)

    # slab free layout per channel partition: (row~, t, w~) flat = 6912
    slab_d = nc.dram_tensor("slab", [C, SLAB_R, S, SLAB_W], bf16, kind="ExternalInput")
    xt_d = nc.dram_tensor("xt", [128, 8, HW], bf16, kind="ExternalInput")
    weff_d = nc.dram_tensor("weff", [C, NTAP], bf16, kind="ExternalInput")
    w1vr_d = nc.dram_tensor("w1vr", [128, 8, C], bf16, kind="ExternalInput")
    hsel_d = nc.dram_tensor("hsel", [128, ROWS], bf16, kind="ExternalInput")
    o_d = nc.dram_tensor("o", [C, HW], f32, kind="ExternalOutput")

    # scratch DRAM for partition-crossing rearrangements
    td_d = nc.dram_tensor("td", [NTAP, SLAB_R, S, SLAB_W], bf16)  # T, tap-major
    ed_d = nc.dram_tensor("ed", [S, ROWS, W], bf16)               # e, (t,row,w)
    dend_d = nc.dram_tensor("dend", [ROWS, W], bf16)              # 1/den

    rr3 = (None, None, None)

    with tile.TileContext(nc) as tc:
        rr3 = (nc.sync, nc.scalar, nc.gpsimd)
        with tc.tile_pool(name="sb", bufs=1) as sb, tc.tile_pool(
            name="ps", bufs=3, space="PSUM"
        ) as ps, tc.tile_pool(name="pso", bufs=1, space="PSUM") as pso:
            # --- loads: weff + slab row 0/1 first so matmuls start early ---
            s_slab = sb.tile([C, SLAB_R, S, SLAB_W], bf16)
            s_weff = sb.tile([C, NTAP], bf16)
            s_hsel = sb.tile([128, ROWS], bf16)
            s_xt = sb.tile([128, 8, HW], bf16)
            s_w1vr = sb.tile([128, 8, C], bf16)
            nc.sync.dma_start(s_slab[:, 0:1], slab_d.ap()[:, 0:1])
            nc.scalar.dma_start(s_weff, weff_d.ap())
            nc.gpsimd.dma_start(s_hsel, hsel_d.ap())
            nc.scalar.dma_start(s_slab[:, 1:2], slab_d.ap()[:, 1:2])
            for i, rp in enumerate(((2, 4), (4, 6), (6, 8), (8, 10), (10, 12))):
                rr3[i % 3].dma_start(
                    s_slab[:, rp[0] : rp[1]], slab_d.ap()[:, rp[0] : rp[1]]
                )
            nc.sync.dma_start(s_xt[:, 0:4, :], xt_d.ap()[:, 0:4, :])
            nc.scalar.dma_start(s_xt[:, 4:8, :], xt_d.ap()[:, 4:8, :])
            nc.gpsimd.dma_start(s_w1vr, w1vr_d.ap())

            # --- phase 1: T[tap, (row~, t, w~)] = weff^T @ slab ---
            # 14 matmuls (13x512 + 256) into 2-bank PSUM tiles, 7 cast copies
            s_T2 = sb.tile([NTAP, SLAB_R, S, SLAB_W], bf16)
            slab_flat = s_slab[:].rearrange("c a b d -> c (a b d)")
            t2_flat = s_T2[:].rearrange("k a b d -> k (a b d)")
            copy_engs = (nc.vector, nc.scalar)
            for pi in range(7):
                f0 = pi * 2 * CH
                f1 = min(f0 + 2 * CH, NPOS)
                p_t = ps.tile([NTAP, 2 * CH], f32, tag="pt")
                for mi in range(2):
                    m0, m1 = f0 + mi * CH, min(f0 + (mi + 1) * CH, NPOS)
                    nc.tensor.matmul(
                        p_t[:, m0 - f0 : m1 - f0],
                        s_weff,
                        slab_flat[:, m0:m1],
                        start=True,
                        stop=True,
                    )
                eng = copy_engs[pi % 2]
                if eng is nc.scalar:
                    eng.copy(t2_flat[:, f0:f1], p_t[:, 0 : f1 - f0])
                else:
                    eng.tensor_copy(t2_flat[:, f0:f1], p_t[:, 0 : f1 - f0])

            # --- T to DRAM (6 row-pair writes) + 10 dy/row-half gathers,
            # interleaved in readiness order so gathers pipeline early.
            # td addr of elem (tap=(5dy+dx), row+dy, t, dx+w)
            #   = dy*(5*NPOS + RT) + dx*(NPOS + 1) + 36*(16*row + t) + w
            s_R = sb.tile([128, NTAP, W], bf16)
            emit_i = [0]

            def emit_td(ci):
                rr3[emit_i[0] % 3].dma_start(
                    td_d.ap()[:, 2 * ci : 2 * ci + 2], s_T2[:, 2 * ci : 2 * ci + 2]
                )
                emit_i[0] += 1

            def emit_gather(dy, half):
                src = bass.AP(
                    tensor=td_d.ap().tensor,
                    offset=dy * (KS * NPOS + RT) + half * 4 * RT,
                    ap=[[SLAB_W, 64], [NPOS + 1, KS], [1, W]],
                )
                rr3[emit_i[0] % 3].dma_start(
                    s_R[64 * half : 64 * half + 64, dy * KS : (dy + 1) * KS], src
                )
                emit_i[0] += 1

            emit_td(0)
            emit_td(1)
            emit_gather(0, 0)
            emit_td(2)
            emit_gather(1, 0)
            emit_gather(2, 0)
            emit_td(3)
            emit_gather(3, 0)
            emit_gather(4, 0)
            emit_gather(0, 1)
            emit_td(4)
            emit_gather(1, 1)
            emit_gather(2, 1)
            emit_td(5)
            emit_gather(3, 1)
            emit_gather(4, 1)

            # --- softmax in row-halves: tap-reduce, exp, e -> DRAM (t,row,w),
            # one 0-stride-replicated read per half rebroadcasts to 128p ---
            s_sk = sb.tile([128, W], f32)   # [(row,t), w]
            s_e = sb.tile([128, W], bf16)
            s_eb = sb.tile([128, HW], bf16)
            p_den = pso.tile([ROWS, W], f32, tag="den")
            for half in range(2):
                p0, p1 = 64 * half, 64 * half + 64
                nc.vector.tensor_reduce(
                    s_sk[p0:p1],
                    s_R[p0:p1].transpose([0, 2, 1]),
                    axis=mybir.AxisListType.X,
                    op=ALU.add,
                )
                nc.scalar.activation(s_e[p0:p1], s_sk[p0:p1], AF.Exp)
                # write e rows (4 rows of this half) into ed[t, row, w]
                dst = bass.AP(
                    tensor=ed_d.ap().tensor,
                    offset=half * 4 * W,
                    ap=[[W, 4], [ROWS * W, S], [1, W]],
                )
                (nc.sync, nc.gpsimd)[half].dma_start(dst, s_e[p0:p1])
                nc.tensor.matmul(
                    p_den,
                    s_hsel[p0:p1],
                    s_e[p0:p1],
                    start=(half == 0),
                    stop=(half == 1),
                )
                # read back broadcast over ci8 groups: dest [:, half*128:+128]
                src = bass.AP(
                    tensor=ed_d.ap().tensor,
                    offset=half * 4 * W,
                    ap=[[0, 8], [ROWS * W, S], [1, 4 * W]],
                )
                (nc.sync, nc.gpsimd)[half].dma_start(
                    s_eb[:, half * 4 * W : half * 4 * W + 4 * W], src
                )

            s_rcp = sb.tile([ROWS, W], bf16)
            with nc.allow_low_precision(reason="1/den fits bf16; tol is 2e-2"):
                nc.vector.reciprocal(s_rcp, p_den)
            nc.scalar.dma_start(dend_d.ap(), s_rcp)
            s_rcpb = sb.tile([C, HW], bf16)
            nc.scalar.dma_start(
                s_rcpb,
                bass.AP(tensor=dend_d.ap().tensor, offset=0, ap=[[0, C], [1, HW]]),
            )

            # --- V path: xattn = x_t * e; contract (ci,t) on PE ---
            s_xa = sb.tile([128, 8, HW], bf16)
            p_o = pso.tile([C, HW], f32, tag="out")
            for half in range(2):
                g0, g1 = 4 * half, 4 * half + 4
                nc.vector.tensor_tensor(
                    s_xa[:, g0:g1, :],
                    s_xt[:, g0:g1, :],
                    s_eb.unsqueeze(1).broadcast_to((128, 4, HW)),
                    op=ALU.mult,
                )
                for g in range(g0, g1):
                    nc.tensor.matmul(
                        p_o,
                        s_w1vr[:, g, :],
                        s_xa[:, g, :],
                        start=(g == 0),
                        stop=(g == 7),
                    )
            # normalize on the PSUM->SBUF read
            s_o = sb.tile([C, HW], f32)
            nc.vector.tensor_tensor(s_o, p_o, s_rcpb, op=ALU.mult)
            nc.sync.dma_start(o_d.ap()[0:32], s_o[0:32])
            nc.scalar.dma_start(o_d.ap()[32:64], s_o[32:64])

    nc.compile()
    return nc


def _get_module():
    global _MODULE
    if _MODULE is None:
        _MODULE = _build_module()
    return _MODULE


def make_host_inputs(x, w1, b1, w2, b2):
    """Host-side precompute: folded weights + per-core reflect-padded slices."""
    x = np.ascontiguousarray(np.asarray(x, np.float32))
    w1 = np.asarray(w1, np.float32)
    w2 = np.asarray(w2, np.float32)

    w1K = w1[C : 2 * C, :, 0, 0]          # [c, ci]
    w2K = w2[0, C : 2 * C]                # [c, 5, 5]
    weff = np.ascontiguousarray(
        np.einsum("ci,cyx->iyx", w1K, w2K).reshape(C, NTAP)
    ).astype(BF16)
    w1V = w1[2 * C :, :, 0, 0]            # [co, ci]

    # w1vr[(ci8,t), g, co] = w1V[co, 8g+ci8]
    tmp = w1V.T.reshape(8, 8, C)                      # (g, ci8, co)
    w1vr = np.ascontiguousarray(
        np.broadcast_to(tmp[:, :, None, :], (8, 8, S, C))
        .transpose(1, 2, 0, 3)
        .reshape(128, 8, C)
    ).astype(BF16)

    # hsel[(row,t), m] = 1 if row == m  (partition index = row*S + t)
    hsel = np.zeros((128, ROWS), np.float32)
    for r in range(ROWS):
        hsel[r * S : (r + 1) * S, r] = 1.0
    hsel = hsel.astype(BF16)

    in_maps = []
    for core in range(NCORES):
        b, hc = divmod(core, 4)
        h0 = ROWS * hc
        xp = np.pad(x[b], ((0, 0), (PAD, PAD), (PAD, PAD), (0, 0)), mode="reflect")
        # slab[c, row~, t, w~]
        slab = np.ascontiguousarray(
            xp[:, h0 : h0 + SLAB_R, :, :].transpose(0, 1, 3, 2)
        ).astype(BF16)
        xs = x[b][:, h0 : h0 + ROWS, :, :]            # [ci, h, w, t]
        xt = np.ascontiguousarray(
            xs.reshape(8, 8, ROWS, W, S)
            .transpose(1, 4, 0, 2, 3)
            .reshape(128, 8, HW)
        ).astype(BF16)
        in_maps.append(
            {"slab": slab, "xt": xt, "weff": weff, "w1vr": w1vr, "hsel": hsel}
        )
    return in_maps


def assemble_output(results, b1):
    b1V = np.asarray(b1, np.float32)[2 * C :]
    out = np.empty((B, C, H, W, S), np.float32)
    for core in range(NCORES):
        b, hc = divmod(core, 4)
        h0 = ROWS * hc
        o = results[core]["o"].reshape(C, ROWS, W, 1)
        out[b, :, h0 : h0 + ROWS, :, :] = o
    out += b1V[None, :, None, None, None]
    return out


def kernel(x, w1, b1, w2, b2):
    from concourse.bass_utils import run_bass_kernel_spmd

    nc = _get_module()
    in_maps = make_host_inputs(x, w1, b1, w2, b2)
    res = run_bass_kernel_spmd(nc, in_maps, core_ids=list(range(NCORES)))
    return assemble_output(res.results, b1)


# revision 10
# speedup vs baseline: 1.4112x; 1.4112x over previous
"""Trainium2 Bass kernel for nn_ConvAttention_34600256537137.

Math notes (validated against the reference):
  qkv = 1x1conv(x, w1)+b1 -> Q,K,V;  score = conv5x5(Q_s)+conv5x5(K_t)+b2;
  attn = softmax_t(score);  out = einsum(attn, V).
  Softmax over t is shift-invariant, so the Q-half of the score (constant in
  t), b2, and the K-path bias all cancel.  The computation collapses to:
    weff[ci,dy,dx] = sum_c w1K[c,ci] * w2K[c,dy,dx]        (host, tiny)
    sK[b,t,h,w]    = conv5x5_reflect(x[b,:,:,:,t], weff)
    e = exp(sK);  den = sum_t e
    out[b,o,h,w,s] = (sum_{ci,t} w1V[o,ci] * e * x) / den + b1V[o]
  (s-independent; normalization + S-broadcast + bias done on host)

Sharding: 8 cores = (b in {0,1}) x (4 chunks of 8 rows of H).

Perf structure (v6):
  - bf16 end-to-end on device (PSUM accumulates fp32); 4x PE rate vs fp32.
  - phase 1 (score conv): T[tap, pos] = weff^T @ slab in 14 contiguous
    512-col matmuls; plain contiguous PSUM->SBUF cast copies alternate
    DVE/Act; slab free layout (row~, t, w~).
  - T bounced to DRAM in [tap, row~, t, w~] layout: the (row,t) partition
    pair merges to a single stride-36 dim, dy folds into the DMA offset and
    dx becomes a stride-(NPOS+1) dim -> the 25 shift-gathers collapse to
    5 DMAs (one per dy).  All td writes + gathers are issued on sync/gpsimd
    only (scalar is busy with conv copies) in readiness order, so the
    in-order queues never head-of-line block.
  - softmax: DVE tap-reduce on 128 lanes, exp on Act; e bounced to DRAM in
    (t,row,w) order so ONE strided read with a 0-stride ci8-replication dim
    rebroadcasts e to all 128 partitions.
  - denominator via indicator-matmul, reciprocal on DVE, 1/den rebroadcast
    via tiny DRAM bounce on scalar (off critical path).
  - V path: xattn = x_t * e_b (DVE, 2 chunks); contract (ci,t) on PE in 8
    bf16 matmuls; normalization folded into the PSUM->SBUF read, split in
    partition halves so output DMAs start early.
  - output is [C, HW] only (S-broadcast on host): 64KB instead of 1MB.
"""

import sys

if "/opt/trn_rl_repo" not in sys.path:
    sys.path.insert(0, "/opt/trn_rl_repo")

import numpy as np
import ml_dtypes

BF16 = ml_dtypes.bfloat16

B, C, H, W, S = 2, 64, 32, 32, 16
KS, PAD = 5, 2
NCORES = 8
ROWS = H // 4            # output rows per core
SLAB_R = ROWS + 2 * PAD  # 12
SLAB_W = W + 2 * PAD     # 36
NTAP = KS * KS           # 25
NPOS = SLAB_R * SLAB_W * S  # 6912 slab positions per tap
HW = ROWS * W            # 256 output positions
RT = SLAB_W * S          # 576 = one row~'s (t, w~) block
CH = 512                 # matmul chunk (free cols); 6912 = 13*512 + 256
NCH = 14

_MODULE = None


def _build_module():
    import concourse.bacc as bacc
    import concourse.bass as bass
    import concourse.tile as tile
    from concourse import mybir

    f32 = mybir.dt.float32
    bf16 = mybir.dt.bfloat16
    AF = mybir.ActivationFunctionType
    ALU = mybir.AluOpType
    nc = bacc.Bacc("TRN2", target_bir_lowering=False, debug=False, num_devices=NCORES)

    # slab free layout per channel partition: (row~, t, w~) flat = 6912
    slab_d = nc.dram_tensor("slab", [C, SLAB_R, S, SLAB_W], bf16, kind="ExternalInput")
    xt_d = nc.dram_tensor("xt", [128, 8, HW], bf16, kind="ExternalInput")
    weff_d = nc.dram_tensor("weff", [C, NTAP], bf16, kind="ExternalInput")
    w1vr_d = nc.dram_tensor("w1vr", [128, 8, C], bf16, kind="ExternalInput")
    hsel_d = nc.dram_tensor("hsel", [128, ROWS], bf16, kind="ExternalInput")
    o_d = nc.dram_tensor("o", [C, HW], f32, kind="ExternalOutput")

    # scratch DRAM for partition-crossing rearrangements
    td_d = nc.dram_tensor("td", [NTAP, SLAB_R, S, SLAB_W], bf16)  # T, tap-major
    ed_d = nc.dram_tensor("ed", [S, ROWS, W], bf16)               # e, (t,row,w)
    dend_d = nc.dram_tensor("dend", [ROWS, W], bf16)              # 1/den

    with tile.TileContext(nc) as tc:
        with tc.tile_pool(name="sb", bufs=1) as sb, tc.tile_pool(
            name="ps", bufs=6, space="PSUM"
        ) as ps, tc.tile_pool(name="pso", bufs=1, space="PSUM") as pso:
            # --- loads: weff + slab rows 0/1 first so matmuls start early ---
            s_slab = sb.tile([C, SLAB_R, S, SLAB_W], bf16)
            s_weff = sb.tile([C, NTAP], bf16)
            s_hsel = sb.tile([128, ROWS], bf16)
            s_xt = sb.tile([128, 8, HW], bf16)
            s_w1vr = sb.tile([128, 8, C], bf16)
            nc.sync.dma_start(s_slab[:, 0:1], slab_d.ap()[:, 0:1])
            nc.scalar.dma_start(s_weff, weff_d.ap())
            nc.gpsimd.dma_start(s_hsel, hsel_d.ap())
            nc.scalar.dma_start(s_slab[:, 1:2], slab_d.ap()[:, 1:2])
            nc.sync.dma_start(s_slab[:, 2:4], slab_d.ap()[:, 2:4])
            nc.scalar.dma_start(s_slab[:, 4:6], slab_d.ap()[:, 4:6])
            nc.gpsimd.dma_start(s_slab[:, 6:8], slab_d.ap()[:, 6:8])
            nc.sync.dma_start(s_slab[:, 8:10], slab_d.ap()[:, 8:10])
            nc.scalar.dma_start(s_slab[:, 10:12], slab_d.ap()[:, 10:12])
            nc.sync.dma_start(s_xt[:, 0:4, :], xt_d.ap()[:, 0:4, :])
            nc.scalar.dma_start(s_xt[:, 4:8, :], xt_d.ap()[:, 4:8, :])
            nc.gpsimd.dma_start(s_w1vr, w1vr_d.ap())

            # --- phase 1: T[tap, (row~, t, w~)] = weff^T @ slab ---
            # 14 matmuls (13x512 + 256) into 1-bank PSUM tiles, cast copies
            s_T2 = sb.tile([NTAP, SLAB_R, S, SLAB_W], bf16)
            slab_flat = s_slab[:].rearrange("c a b d -> c (a b d)")
            t2_flat = s_T2[:].rearrange("k a b d -> k (a b d)")
            copy_engs = (nc.vector, nc.scalar)
            for mi in range(NCH):
                f0 = mi * CH
                f1 = min(f0 + CH, NPOS)
                p_t = ps.tile([NTAP, CH], f32, tag="pt")
                nc.tensor.matmul(
                    p_t[:, 0 : f1 - f0],
                    s_weff,
                    slab_flat[:, f0:f1],
                    start=True,
                    stop=True,
                )
                eng = copy_engs[mi % 2]
                if eng is nc.scalar:
                    eng.copy(t2_flat[:, f0:f1], p_t[:, 0 : f1 - f0])
                else:
                    eng.tensor_copy(t2_flat[:, f0:f1], p_t[:, 0 : f1 - f0])

            # --- T to DRAM (6 row-pair writes) then 5 dy-gathers, all on
            # sync/gpsimd (scalar is busy with conv copies), readiness order.
            # td addr of elem (tap=(5dy+dx), row+dy, t, dx+w)
            #   = dy*(5*NPOS + RT) + dx*(NPOS + 1) + 36*(16*row + t) + w
            s_R = sb.tile([128, NTAP, W], bf16)
            for ci in range(6):
                (nc.sync, nc.gpsimd)[ci % 2].dma_start(
                    td_d.ap()[:, 2 * ci : 2 * ci + 2], s_T2[:, 2 * ci : 2 * ci + 2]
                )
            for dy in range(KS):
                src = bass.AP(
                    tensor=td_d.ap().tensor,
                    offset=dy * (KS * NPOS + RT),
                    ap=[[SLAB_W, 128], [NPOS + 1, KS], [1, W]],
                )
                (nc.sync, nc.gpsimd)[dy % 2].dma_start(
                    s_R[:, dy * KS : (dy + 1) * KS], src
                )

            # --- softmax: tap-reduce, exp, e -> DRAM (t,row,w), one
            # 0-stride-replicated read rebroadcasts e to all 128 partitions ---
            s_sk = sb.tile([128, W], f32)   # [(row,t), w]
            s_e = sb.tile([128, W], bf16)
            s_eb = sb.tile([128, HW], bf16)
            nc.vector.tensor_reduce(
                s_sk, s_R.transpose([0, 2, 1]), axis=mybir.AxisListType.X, op=ALU.add
            )
            nc.scalar.activation(s_e, s_sk, AF.Exp)
            nc.sync.dma_start(
                bass.AP(
                    tensor=ed_d.ap().tensor,
                    offset=0,
                    ap=[[W, ROWS], [ROWS * W, S], [1, W]],
                ),
                s_e,
            )
            nc.gpsimd.dma_start(
                s_eb,
                bass.AP(
                    tensor=ed_d.ap().tensor,
                    offset=0,
                    ap=[[0, 8], [ROWS * W, S], [1, HW]],
                ),
            )
            p_den = pso.tile([ROWS, W], f32, tag="den")
            nc.tensor.matmul(p_den, s_hsel, s_e, start=True, stop=True)
            s_rcp = sb.tile([ROWS, W], bf16)
            with nc.allow_low_precision(reason="1/den fits bf16; tol is 2e-2"):
                nc.vector.reciprocal(s_rcp, p_den)
            nc.scalar.dma_start(dend_d.ap(), s_rcp)
            s_rcpb = sb.tile([C, HW], bf16)
            nc.scalar.dma_start(
                s_rcpb,
                bass.AP(tensor=dend_d.ap().tensor, offset=0, ap=[[0, C], [1, HW]]),
            )

            # --- V path: xattn = x_t * e; contract (ci,t) on PE ---
            s_xa = sb.tile([128, 8, HW], bf16)
            p_o = pso.tile([C, HW], f32, tag="out")
            for half in range(2):
                g0, g1 = 4 * half, 4 * half + 4
                nc.vector.tensor_tensor(
                    s_xa[:, g0:g1, :],
                    s_xt[:, g0:g1, :],
                    s_eb.unsqueeze(1).broadcast_to((128, 4, HW)),
                    op=ALU.mult,
                )
                for g in range(g0, g1):
                    nc.tensor.matmul(
                        p_o,
                        s_w1vr[:, g, :],
                        s_xa[:, g, :],
                        start=(g == 0),
                        stop=(g == 7),
                    )
            # normalize on the PSUM->SBUF read; split so out DMAs start early
            s_o = sb.tile([C, HW], f32)
            nc.vector.tensor_tensor(s_o[0:32], p_o[0:32], s_rcpb[0:32], op=ALU.mult)
            nc.sync.dma_start(o_d.ap()[0:32], s_o[0:32])
            nc.vector.tensor_tensor(s_o[32:64], p_o[32:64], s_rcpb[32:64], op=ALU.mult)
            nc.scalar.dma_start(o_d.ap()[32:64], s_o[32:64])

    nc.compile()
    return nc


def _get_module():
    global _MODULE
    if _MODULE is None:
        _MODULE = _build_module()
    return _MODULE


def make_host_inputs(x, w1, b1, w2, b2):
    """Host-side precompute: folded weights + per-core reflect-padded slices."""
    x = np.ascontiguousarray(np.asarray(x, np.float32))
    w1 = np.asarray(w1, np.float32)
    w2 = np.asarray(w2, np.float32)

    w1K = w1[C : 2 * C, :, 0, 0]          # [c, ci]
    w2K = w2[0, C : 2 * C]                # [c, 5, 5]
    weff = np.ascontiguousarray(
        np.einsum("ci,cyx->iyx", w1K, w2K).reshape(C, NTAP)
    ).astype(BF16)
    w1V = w1[2 * C :, :, 0, 0]            # [co, ci]

    # w1vr[(ci8,t), g, co] = w1V[co, 8g+ci8]
    tmp = w1V.T.reshape(8, 8, C)                      # (g, ci8, co)
    w1vr = np.ascontiguousarray(
        np.broadcast_to(tmp[:, :, None, :], (8, 8, S, C))
        .transpose(1, 2, 0, 3)
        .reshape(128, 8, C)
    ).astype(BF16)

    # hsel[(row,t), m] = 1 if row == m  (partition index = row*S + t)
    hsel = np.zeros((128, ROWS), np.float32)
    for r in range(ROWS):
        hsel[r * S : (r + 1) * S, r] = 1.0
    hsel = hsel.astype(BF16)

    in_maps = []
    for core in range(NCORES):
        b, hc = divmod(core, 4)
        h0 = ROWS * hc
        xp = np.pad(x[b], ((0, 0), (PAD, PAD), (PAD, PAD), (0, 0)), mode="reflect")
        # slab[c, row~, t, w~]
        slab = np.ascontiguousarray(
            xp[:, h0 : h0 + SLAB_R, :, :].transpose(0, 1, 3, 2)
        ).astype(BF16)
        xs = x[b][:, h0 : h0 + ROWS, :, :]            # [ci, h, w, t]
        xt = np.ascontiguousarray(
            xs.reshape(8, 8, ROWS, W, S)
            .transpose(1, 4, 0, 2, 3)
            .reshape(128, 8, HW)
        ).astype(BF16)
        in_maps.append(
            {"slab": slab, "xt": xt, "weff": weff, "w1vr": w1vr, "hsel": hsel}
        )
    return in_maps


def assemble_output(results, b1):
    b1V = np.asarray(b1, np.float32)[2 * C :]
    out = np.empty((B, C, H, W, S), np.float32)
    for core in range(NCORES):
        b, hc = divmod(core, 4)
        h0 = ROWS * hc
        o = results[core]["o"].reshape(C, ROWS, W, 1)
        out[b, :, h0 : h0 + ROWS, :, :] = o
    out += b1V[None, :, None, None, None]
    return out


def kernel(x, w1, b1, w2, b2):
    from concourse.bass_utils import run_bass_kernel_spmd

    nc = _get_module()
    in_maps = make_host_inputs(x, w1, b1, w2, b2)
    res = run_bass_kernel_spmd(nc, in_maps, core_ids=list(range(NCORES)))
    return assemble_output(res.results, b1)
